# revision 29
# baseline (speedup 1.0000x reference)
"""Single-head causal attention on 8 NeuronCores (batch-parallel), bf16.

x [8, 2048, 1024], Wq/Wk/Wv [1024, 64] -> out [8, 2048, 64].
One batch element per core. The host pre-transposes x to x.T (chunk-major
layout) and casts everything to bf16 (zero-flop marshalling), so the
device does no transposes at all:

  qkT[:,t]   = [Wq|Wk].T @ xT[:,t]      (qT rows 0:64, kT rows 64:128)
  v[t,:]     = xT[:,t-tile].T @ Wv      (natural [t,h] layout, PE direct)
  weiT[s,t]  = k[s]. q[t]              (lhsT = kT tile, rhs = qT cols)
  pT         = exp(weiT / sqrt(H))      (ACT, f32 psum -> bf16 sbuf,
                                         two s-tiles per instruction)
  out[t,h]   = sum_s pT[s,t] vaug[s,h]  (natural PV; ones column gives
                                         softmax denominators)
  out[t,h]  /= out[t,64]               (DVE reciprocal + scalar mul)

Causality via tile skipping, column-restricted diagonal score matmuls,
and one [128,128] triangular bf16 mask on diagonal blocks.  x.T chunks
are DMA'd through three initiating engines (sync/scalar/gpsimd) so the
first chunk lands early; emission interleaves proj(ch+1) with chunk ch's
PV stream so the scalar engine's exp pipeline never starves.
"""

from contextlib import ExitStack

import ml_dtypes
import numpy as np

import concourse.bass as bass
import concourse.mybir as mybir
import concourse.tile as tile
from concourse import bacc
from concourse.bass_utils import run_bass_kernel_spmd
from concourse.masks import make_upper_triangular

B, T, C, H = 8, 2048, 1024, 64
P = 128                      # partition tile
NT = T // P                  # 16 row tiles
NC = C // P                  # 8 contraction tiles
CH = 512                     # t-chunk width (psum bank)
NCH = T // CH                # 4 chunks
TPC = CH // P                # 4 t-tiles per chunk
VW = 66                      # vaug row stride: [v(64) | 1 | pad]

BF = mybir.dt.bfloat16
F32 = mybir.dt.float32
BF_NP = ml_dtypes.bfloat16

Exp = mybir.ActivationFunctionType.Exp


def build_kernel():
    nc = bacc.Bacc(
        "TRN2",
        target_bir_lowering=False,
        debug=False,
        enable_asserts=False,
        num_devices=B,
    )
    xTd = nc.dram_tensor("xT", [NCH, P, NC, CH], BF, kind="ExternalInput").ap()
    wqkd = nc.dram_tensor("Wqk", [P, NC, P], BF, kind="ExternalInput").ap()
    wvd = nc.dram_tensor("Wv", [P, NC, H], BF, kind="ExternalInput").ap()
    outd = nc.dram_tensor("out", [T, H], F32, kind="ExternalOutput").ap()

    with tile.TileContext(nc) as tc, ExitStack() as ctx:
        const = ctx.enter_context(tc.tile_pool(name="const", bufs=1))
        persist = ctx.enter_context(tc.tile_pool(name="persist", bufs=1))
        pt_p = ctx.enter_context(tc.tile_pool(name="pt", bufs=20))
        ost_p = ctx.enter_context(tc.tile_pool(name="ost", bufs=4))
        rc_p = ctx.enter_context(tc.tile_pool(name="rc", bufs=4))
        proj_ps = ctx.enter_context(tc.tile_pool(name="projps", bufs=2, space="PSUM"))
        wei_ps = ctx.enter_context(tc.tile_pool(name="weips", bufs=2, space="PSUM"))
        o_ps_p = ctx.enter_context(tc.tile_pool(name="ops", bufs=2, space="PSUM"))

        # x.T: chunks split across scalar+gpsimd DMA queues (sync reserved for
        # small latency-critical transfers: wqk, kTlo, out).  The DMA engines
        # round-robin across ALL enqueued transfers, so chunk ch+1 is only
        # enqueued once chunk ch has landed (1-element dummy reads gate the
        # sequencers); otherwise chunk 0's tail packets finish last and the
        # whole pipeline start slips by ~7us.
        # PE p-state warmup: matmuls on a zeroed scratch tile (no DMA
        # dependency) keep the tensor engine running during the x DMA wait so
        # the clock has ramped to 2.4 GHz before the first real projection.
        garbage = const.tile([P, CH], BF, tag="garbage")
        nc.vector.memset(garbage, 0.0)
        warm_ps = proj_ps.tile([P, CH], F32, tag="ps")
        for _ in range(9):
            nc.tensor.matmul(warm_ps, garbage[:, 0:P], garbage, start=True, stop=True)

        # x load plan.  Facts: DMA engines round-robin across ALL enqueued
        # transfers, so anything queued beside chunk 0 delays it; a WAR-gated
        # dma_start blocks its host sequencer until the gate fires, so gated
        # pieces may only live on sync/gpsimd (scalar runs the exp stream).
        # Shallow gates (chunk2 on chunk0, chunk3 on chunk1) trade a little
        # contention for early issue - measured faster than a strict chain.
        #   sync:   wqk, chunk0[c6:8], chunk1, chunk2[c0:4]*, chunk3[c0:4]*,
        #           kTlo1..3   (* = WAR-gated)
        #   scalar: wv, chunk0[c0:3]
        #   gpsimd: chunk0[c3:6], gate writes, chunk2[c4:8]*, chunk3[c4:8]*,
        #           out stores
        xT = persist.tile([P, NC, T], BF, tag="xT")  # x.T: [c, t]
        wqk = const.tile([P, NC, P], BF, tag="wqk")
        nc.sync.dma_start(wqk, wqkd)
        wv = const.tile([P, NC, H], BF, tag="wv")
        nc.scalar.dma_start(wv, wvd)
        nc.scalar.dma_start(xT[:, 0:3, 0:CH], xTd[0][:, 0:3, :])
        nc.gpsimd.dma_start(xT[:, 3:6, 0:CH], xTd[0][:, 3:6, :])
        nc.sync.dma_start(xT[:, 6:8, 0:CH], xTd[0][:, 6:8, :])
        nc.sync.dma_start(xT[:, :, CH : 2 * CH], xTd[1])
        for ch in (2, 3):
            c0 = ch * CH
            g0 = 0 if ch == 2 else CH  # gate chunk2 on chunk0, chunk3 on chunk1
            nc.gpsimd.tensor_copy(
                xT[0:1, :, c0 : c0 + 1],
                xT[0:1, 0, g0 + CH - 1 : g0 + CH].broadcast_to((1, NC, 1)),
            )
            nc.sync.dma_start(xT[:, 0:4, c0 : c0 + CH], xTd[ch][:, 0:4, :])
            nc.gpsimd.dma_start(xT[:, 4:8, c0 : c0 + CH], xTd[ch][:, 4:8, :])

        # causal mask: gpsimd writes f32; DVE copy converts to bf16
        scr_t = const.tile([P, P], F32, tag="scr_t")
        make_upper_triangular(nc, scr_t, val=1.0, diag=True)
        tri = const.tile([P, P], BF, tag="tri")  # tri[p,j]=1 iff j>=p
        nc.vector.tensor_copy(tri, scr_t)

        qkT = persist.tile([P, T], BF, tag="qkT")    # qT rows 0:64, kT 64:128
        kTlo = persist.tile([H, T], BF, tag="kTlo")  # kT re-based at partition 0
        vaug = persist.tile([P, NT, VW], BF, tag="vaug")  # [v | 1] per s-tile
        ones = nc.const_aps.scalar_like(1.0, vaug)
        nc.vector.tensor_copy(vaug[:, :, H : H + 1], ones.broadcast_to((P, NT, 1)))

        def proj_qk(ch):
            chs = slice(ch * CH, (ch + 1) * CH)
            qk_ps = proj_ps.tile([P, CH], F32, tag="ps")
            for c in range(NC):
                nc.tensor.matmul(
                    qk_ps, wqk[:, c, :], xT[:, c, chs], start=(c == 0), stop=(c == NC - 1)
                )
            nc.vector.tensor_copy(qkT[0:P if ch else H, chs], qk_ps[0:P if ch else H, :])
            if ch == 0:
                # chunk 0's attention starts immediately after this projection;
                # an extra PE k-projection into partitions 0:64 (two halves, so
                # the first score pair starts after half of it) avoids waiting
                # on the SBUF->SBUF rebasing DMA's ~2.5us latency.
                k0_ps = proj_ps.tile([H, CH], F32, tag="ps", name="k0_ps")
                for c in range(NC):
                    nc.tensor.matmul(
                        k0_ps, wqk[:, c, H:P], xT[:, c, chs],
                        start=(c == 0), stop=(c == NC - 1),
                    )
                nc.vector.tensor_copy(kTlo[:, chs], k0_ps)
            else:
                nc.sync.dma_start(kTlo[:, chs], qkT[H:P, chs])

        def proj_v(ch):
            v_ps = proj_ps.tile([P, TPC, H], F32, tag="ps")
            for j in range(TPC):
                s = TPC * ch + j
                for c in range(NC):
                    nc.tensor.matmul(
                        v_ps[:, j, :],
                        xT[:, c, s * P : (s + 1) * P],
                        wv[:, c, :],
                        start=(c == 0),
                        stop=(c == NC - 1),
                    )
            nc.vector.tensor_copy(vaug[:, TPC * ch : TPC * ch + TPC, 0:H], v_ps)

        def emit_scores(ch, i):
            """Score matmuls + merged exp for s-tile pair (2i, 2i+1) of chunk ch."""
            base = ch * CH
            wei = wei_ps.tile([P, 2, CH], F32, tag="w")
            cols = []
            for u in range(2):
                s = 2 * i + u
                diag = s >= TPC * ch
                col0 = (s - TPC * ch) * P if diag else 0
                cols.append(col0)
                nc.tensor.matmul(
                    wei[:, u, col0:],
                    kTlo[:, s * P : (s + 1) * P],
                    qkT[0:H, base + col0 : base + CH],
                    start=True,
                    stop=True,
                )
            cmin = min(cols)
            pT = pt_p.tile([P, 2, CH], BF)
            # one ACT instruction covers both s-tiles; cols [cmin:col0) of a
            # diagonal tile hold exp(stale psum) — finite and never read.
            nc.scalar.activation(
                pT[:, :, cmin:], wei[:, :, cmin:], Exp, scale=float(H) ** -0.5
            )
            for u in range(2):
                s = 2 * i + u
                if s >= TPC * ch:
                    c0 = cols[u]
                    nc.vector.tensor_mul(
                        pT[:, u, c0 : c0 + P], pT[:, u, c0 : c0 + P], tri
                    )
            return [(2 * i, cols[0], pT, 0), (2 * i + 1, cols[1], pT, 1)]

        def emit_pv(ch, o_ps, s, col0, pT, u):
            # start=True clears has_written for the WHOLE psum bank, so only
            # the first matmul of the chunk may set it; later slices' first
            # writes land on cleared bits and overwrite, then accumulate.
            for j in range(col0 // P, TPC):
                tj = TPC * ch + j
                nc.tensor.matmul(
                    o_ps[:, j, :],
                    pT[:, u, j * P : (j + 1) * P],
                    vaug[:, s, 0 : H + 1],
                    start=(s == 0 and j == 0),
                    stop=(s == tj),
                    skip_group_check=True,
                )


        def epilogue(ch, o_ps):
            rc = rc_p.tile([P, TPC, 1], F32)
            nc.vector.reciprocal(rc, o_ps[:, :, H : H + 1])
            ost = ost_p.tile([P, TPC, H], F32)
            for j in range(TPC):
                nc.vector.tensor_scalar_mul(ost[:, j, :], o_ps[:, j, 0:H], rc[:, j, :])
            nc.gpsimd.dma_start(
                outd[ch * CH : (ch + 1) * CH, :].rearrange("(n p) h -> p n h", p=P), ost
            )

        # Emission: scores/exp stream ahead, PV of chunk ch interleaves with
        # proj of chunk ch+1 so neither PE nor ACT starves.
        o_ps = {}
        pend = []

        def drain(n):
            while len(pend) > n:
                ch_, s_, c0_, pT_, u_ = pend.pop(0)
                emit_pv(ch_, o_ps[ch_], s_, c0_, pT_, u_)
                if s_ == TPC * ch_ + TPC - 1:
                    epilogue(ch_, o_ps.pop(ch_))

        proj_qk(0)
        for ch in range(NCH):
            o_ps[ch] = o_ps_p.tile([P, TPC, H + 1], F32, tag="o", name=f"o_ps{ch}")
            npairs = (TPC * ch + TPC) // 2
            for i in range(npairs):
                for e in emit_scores(ch, i):
                    pend.append((ch, *e))
                if i == 0:
                    proj_v(ch)
                if i == (1 if ch == 0 else 2 * ch) and ch + 1 < NCH:
                    proj_qk(ch + 1)
                drain(3)
        drain(0)

    nc.compile()
    return nc


_NC = None


def kernel(x, Wq, Wk, Wv, **run_kwargs):
    global _NC
    if _NC is None:
        _NC = build_kernel()
    x = np.asarray(x, dtype=np.float32)
    wqk = np.concatenate(
        [np.asarray(Wq, np.float32), np.asarray(Wk, np.float32)], axis=1
    ).astype(BF_NP)
    wqk_t = np.ascontiguousarray(wqk.reshape(NC, P, P).transpose(1, 0, 2))
    wv_t = np.ascontiguousarray(
        np.asarray(Wv, np.float32).astype(BF_NP).reshape(NC, P, H).transpose(1, 0, 2)
    )
    in_maps = []
    for b in range(B):
        xT = x[b].T.astype(BF_NP)  # [C, T]
        # chunk-major tiled layout: [NCH, P, NC, CH], 8 KiB contiguous lines
        xT_t = np.ascontiguousarray(
            xT.reshape(NC, P, NCH, CH).transpose(2, 1, 0, 3)
        )
        in_maps.append({"xT": xT_t, "Wqk": wqk_t, "Wv": wv_t})
    res = run_bass_kernel_spmd(_NC, in_maps, core_ids=list(range(B)), **run_kwargs)
    out = np.stack([res.results[b]["out"] for b in range(B)])
    if run_kwargs:
        kernel.last_result = res
    return out


if __name__ == "__main__":
    rng = np.random.default_rng(0)
    ins = {
        "x": rng.standard_normal((B, T, C), dtype=np.float32),
        "Wq": rng.standard_normal((C, H), dtype=np.float32) / np.sqrt(C),
        "Wk": rng.standard_normal((C, H), dtype=np.float32) / np.sqrt(C),
        "Wv": rng.standard_normal((C, H), dtype=np.float32) / np.sqrt(C),
    }
    out = kernel(**ins)
    print("out", out.shape, out.dtype)
